# revision 1
# baseline (speedup 1.0000x reference)
"""DTNN layer kernel for Trainium2 (8 NeuronCores).

Math: out[b,i,o] = sum_j sum_h Wfc[o,h] * hx[b,i,h] * hd[b,i,j,h]
with hx = x@Wcf.T + bcf, hd = dist@Wdf.T + bdf.
Since Wfc/Wdf are linear, the j-sum commutes:
    ds[b,i,d]  = sum_j dist[b,i,j,d]                  (memory-bound reduction)
    out[b,i,:] = ((x@Wcf.T + bcf) * (ds@Wdf.T + N*bdf)) @ Wfc.T
So the kernel streams `distance` once (134MB) and does a few 128x128 matmuls.

Sharding: flatten (B,N) -> 1024 i-rows, 128 rows per core; no cross-core comms.

Measured (NTFF profile, core 0): ~70us/core, vs ~47us pure HBM stream at the
358 GB/s per-core fair share plus ~13us fixed NEFF prologue/epilogue and a
~9us serial tail. Structure:
- dist is streamed as a few big HWDGE DMAs on one ring (in-order arrivals);
  DVE folds each tile to 128 columns in place right after it lands (halving
  unit-stride adds run at full DVE rate; strided reduces were 1.6x slower).
- biases are folded into PE matmuls as K=1 rank-1 updates, and the
  (hx * N*bdf) @ WfcT bias term is preloaded into the output PSUM during the
  stream so the post-stream tail is just transpose -> Wdf matmul -> mul ->
  accumulate-matmul -> store.
"""

import numpy as np

import concourse.bass as bass
import concourse.bacc as bacc
import concourse.mybir as mybir
from concourse.tile import TileContext
from concourse.bass_utils import run_bass_kernel_spmd

B, N, D, H = 4, 256, 128, 128
NCORES = 8
ROWS = B * N // NCORES  # 128 i-rows per core
FP = mybir.dt.float32

# packed constant columns: [xT | wcfT | wdfT | wfcT | eye | rows...]
C_XT = 0
C_WCF = 128
C_WDF = 256
C_WFC = 384
C_EYE = 512
C_BCFR = 640   # partition 0: bcf row (1, H)
C_BDFR = 768   # partition 0: bdf row (1, H)
C_ONES = 896   # partition 0: ones row (1, ROWS)
C_BDFC = 1024  # bdf as a per-partition column (H, 1)
C_TOT = 1025


def build_nc():
    nc = bacc.Bacc("TRN2", target_bir_lowering=False)
    dist = nc.declare_dram_parameter("dist", [ROWS, N * D], FP, isOutput=False)
    cst = nc.declare_dram_parameter("cst", [128, C_TOT], FP, isOutput=False)
    out = nc.declare_dram_parameter("out", [ROWS, D], FP, isOutput=True)

    with TileContext(nc) as tc:
        with (
            tc.tile_pool(name="const", bufs=1) as cpool,
            tc.tile_pool(name="dist", bufs=1) as dpool,
            tc.tile_pool(name="work", bufs=1) as wpool,
            tc.tile_pool(name="psum", bufs=1, space="PSUM") as ppool,
        ):
            # Issue the dist stream first so the big DMAs start ASAP; the
            # constants ride behind them on the same queue.
            SIZES = [64, 64, 64, 32, 16, 8, 4, 4]  # j-counts per DMA tile
            dtiles = []
            off = 0
            for k, jn in enumerate(SIZES):
                t = dpool.tile([ROWS, jn * D], FP, tag=f"dist{k}")
                # Single HWDGE ring (SP): in-order arrivals matching the DVE
                # fold order; the stream is HBM-fair-share-bound (~358GB/s)
                # so a second ring adds no bandwidth, only ordering jitter.
                nc.sync.dma_start(out=t[:], in_=dist[:, off * D:(off + jn) * D])
                dtiles.append(t)
                off += jn

            cst_t = cpool.tile([128, C_TOT], FP)
            nc.scalar.dma_start(out=cst_t[:], in_=cst[:])
            xT_t = cst_t[:, C_XT:C_XT + ROWS]
            wcf_t = cst_t[:, C_WCF:C_WCF + H]
            wdf_t = cst_t[:, C_WDF:C_WDF + H]
            wfc_t = cst_t[:, C_WFC:C_WFC + D]
            ident = cst_t[:, C_EYE:C_EYE + ROWS]
            bcf_row = cst_t[0:1, C_BCFR:C_BCFR + H]
            ones_row = cst_t[0:1, C_ONES:C_ONES + ROWS]

            # hx^T = (Wcf^T)^T @ x^T + bcf x ones -> (H, ROWS) in PSUM
            hx_ps = ppool.tile([H, ROWS], FP)
            nc.tensor.matmul(hx_ps[:], wcf_t, xT_t, start=True, stop=False)
            nc.tensor.matmul(hx_ps[:], bcf_row, ones_row, start=False, stop=True)
            hxT = wpool.tile([H, ROWS], FP)
            nc.vector.tensor_copy(hxT[:], hx_ps[:])

            # Preload the bias term (hx * N*bdf) @ Wfc^T into the output
            # PSUM during the stream; the tail's out-matmul accumulates
            # onto it, removing the bias matmul from the critical tail.
            bdfN = wpool.tile([H, 1], FP)
            nc.vector.tensor_scalar_mul(bdfN[:], cst_t[:, C_BDFC:C_BDFC + 1],
                                        float(N))
            s0T = wpool.tile([H, ROWS], FP)
            nc.vector.tensor_scalar_mul(s0T[:], hxT[:], bdfN[:])
            out_ps = ppool.tile([ROWS, D], FP)
            nc.tensor.matmul(out_ps[:], s0T[:], wfc_t, start=True, stop=False)

            # Streaming j-reduction: ds[i,d] = sum_j dist[i,j,d].
            # Each tile is folded to 128 columns in place immediately after
            # its DMA lands (halving adds, all unit-stride = full DVE rate),
            # then added into the running accumulator (tile 0). Per-tile DVE
            # work (~4.9us) keeps pace with per-tile DMA arrival (~5.1us),
            # so only ~2us of DVE work remains after the last (half-size)
            # tile arrives.
            acc = dtiles[0]
            for k, jn in enumerate(SIZES):
                t = dtiles[k]
                half = jn * D // 2
                while half >= D:
                    nc.vector.tensor_add(
                        t[:, 0:half], t[:, 0:half], t[:, half:2 * half]
                    )
                    half //= 2
                if k > 0:
                    nc.vector.tensor_add(acc[:, 0:D], acc[:, 0:D], t[:, 0:D])
            ds = acc[:, 0:D]

            # ds (i,d) -> dsT (d,i) via PE transpose
            dsT_ps = ppool.tile([D, ROWS], FP)
            nc.tensor.transpose(dsT_ps[:], ds, ident)
            dsT = wpool.tile([D, ROWS], FP)
            nc.vector.tensor_copy(dsT[:], dsT_ps[:])

            # hd^T (bias-free) = (Wdf^T)^T @ ds^T -> (H, ROWS)
            hd_ps = ppool.tile([H, ROWS], FP)
            nc.tensor.matmul(hd_ps[:], wdf_t, dsT[:], start=True, stop=True)

            # s^T = hx^T * hd^T (one PSUM operand max per DVE op)
            sT = wpool.tile([H, ROWS], FP)
            nc.vector.tensor_mul(sT[:], hd_ps[:], hxT[:])

            # out += sT^T @ Wfc^T, accumulating onto the preloaded bias term
            nc.tensor.matmul(out_ps[:], sT[:], wfc_t, start=False, stop=True,
                             skip_group_check=True)
            out_sb = wpool.tile([ROWS, D], FP)
            nc.vector.tensor_copy(out_sb[:], out_ps[:])
            nc.sync.dma_start(out=out[:], in_=out_sb[:])
    nc.compile()
    return nc


_NC_CACHE = None


def _get_nc():
    global _NC_CACHE
    if _NC_CACHE is None:
        _NC_CACHE = build_nc()
    return _NC_CACHE


def _make_in_maps(x, distance, Wcf_w, Wcf_b, Wdf_w, Wdf_b, Wfc_w):
    x = np.ascontiguousarray(np.asarray(x, np.float32))
    distance = np.ascontiguousarray(np.asarray(distance, np.float32))
    x_flat = x.reshape(B * N, D)
    dist_flat = distance.reshape(B * N, N * D)
    wcfT = np.asarray(Wcf_w, np.float32).T
    wdfT = np.asarray(Wdf_w, np.float32).T
    wfcT = np.asarray(Wfc_w, np.float32).T
    bcf = np.asarray(Wcf_b, np.float32)
    bdf = np.asarray(Wdf_b, np.float32)
    in_maps = []
    for c in range(NCORES):
        sl = slice(c * ROWS, (c + 1) * ROWS)
        cstblk = np.zeros((128, C_TOT), np.float32)
        cstblk[:, C_XT:C_XT + ROWS] = x_flat[sl].T
        cstblk[:, C_WCF:C_WCF + H] = wcfT
        cstblk[:, C_WDF:C_WDF + H] = wdfT
        cstblk[:, C_WFC:C_WFC + D] = wfcT
        cstblk[:, C_EYE:C_EYE + ROWS] = np.eye(ROWS, dtype=np.float32)
        cstblk[0, C_BCFR:C_BCFR + H] = bcf
        cstblk[0, C_BDFR:C_BDFR + H] = bdf
        cstblk[0, C_ONES:C_ONES + ROWS] = 1.0
        cstblk[:, C_BDFC] = bdf
        in_maps.append({
            "dist": np.ascontiguousarray(dist_flat[sl]),
            "cst": cstblk,
        })
    return in_maps


def kernel(x, distance, Wcf_w, Wcf_b, Wdf_w, Wdf_b, Wfc_w):
    in_maps = _make_in_maps(x, distance, Wcf_w, Wcf_b, Wdf_w, Wdf_b, Wfc_w)
    nc = _get_nc()
    res = run_bass_kernel_spmd(nc, in_maps, list(range(NCORES))).results
    out = np.concatenate([res[c]["out"] for c in range(NCORES)], axis=0)
    return out.reshape(B, N, D)



# revision 3
# speedup vs baseline: 1.1401x; 1.1401x over previous
"""DTNN layer kernel for Trainium2 (8 NeuronCores).

Math: out[b,i,o] = sum_j sum_h Wfc[o,h] * hx[b,i,h] * hd[b,i,j,h]
with hx = x@Wcf.T + bcf, hd = dist@Wdf.T + bdf.
Since Wfc/Wdf are linear, the j-sum commutes:
    ds[b,i,d]  = sum_j dist[b,i,j,d]                  (memory-bound reduction)
    out[b,i,:] = ((x@Wcf.T + bcf) * (ds@Wdf.T + N*bdf)) @ Wfc.T
So the kernel streams `distance` once (134MB) and does a few 128x128 matmuls.

Sharding: flatten (B,N) -> 1024 i-rows, 128 rows per core; no cross-core comms.

v2 design (from NTFF trace analysis of v1, which ran ~70-75us):
- v1's fold (DVE halving adds, ~41us busy) lagged the 43.5us DMA stream by
  ~19us because the big 64-j tiles were folded big-first (DVE idle until the
  first 4.2MB tile landed at ~17us) and the j=4 endgame tiles trickled at
  ~100GB/s (2KB per-partition lines expose HBM latency). Serial tail ~5us.
- v2: mostly-32j tiles (16KB lines, full-rate packets) tapering to 8j so the
  fold tracks arrivals; fold mid-stages run in bf16 (2x DVE rate), stage 1
  casts fp32->bf16, last stage emits a 128-col fp32 chunk result.
- Each chunk result is transposed on the (idle) PE with an accumulating
  is_transpose matmul into one PSUM tile: ds^T accumulation is free, no DVE
  acc-adds and no post-stream transpose.
- All PE matmuls run bf16 (1 cycle/row vs 4 for fp32): weights/x/biases are
  pre-cast host-side. Output is computed mirrored (out^T = WfcT^T @ sT) so
  the final matmul's stationary operand is a constant; host transposes back.
- PSUM->SBUF copies ride the Scalar(ACT) engine, keeping DVE for folds only.
Numpy-simulated rel err of this scheme: 4.1e-3 (gate 2e-2).
"""

import numpy as np
from ml_dtypes import bfloat16

import concourse.bass as bass
import concourse.bacc as bacc
import concourse.mybir as mybir
from concourse.tile import TileContext
from concourse.bass_utils import run_bass_kernel_spmd

B, N, D, H = 4, 256, 128, 128
NCORES = 8
ROWS = B * N // NCORES  # 128 i-rows per core
FP = mybir.dt.float32
BF = mybir.dt.bfloat16

# j-counts per streamed tile: 32j tiles (16KB per-partition lines = full-rate
# 16KB DMA packets) tapering to 8j so the last fold chain is short.
SIZES = [32, 32, 32, 32, 32, 32, 16, 16, 16, 8, 8]
assert sum(SIZES) == N

# bf16 constant block columns: [xT | wcfT | wdfT | wfcT | bcf_row | ones_row]
CB_XT = 0
CB_WCF = 128
CB_WDF = 256
CB_WFC = 384
CB_BCFR = 512   # partition 0: bcf row (1, H)
CB_ONES = 640   # partition 0: ones row (1, ROWS)
CB_TOT = 768

# fp32 constant block columns: [eye | N*bdf col]
CF_EYE = 0
CF_BDFN = 128   # per-partition column (H, 1) = N * bdf
CF_TOT = 129


def build_nc():
    nc = bacc.Bacc("TRN2", target_bir_lowering=False)
    dist = nc.declare_dram_parameter("dist", [ROWS, N * D], FP, isOutput=False)
    cstb = nc.declare_dram_parameter("cstb", [128, CB_TOT], BF, isOutput=False)
    cstf = nc.declare_dram_parameter("cstf", [128, CF_TOT], FP, isOutput=False)
    out = nc.declare_dram_parameter("out", [D, ROWS], FP, isOutput=True)

    with TileContext(nc) as tc:
        with (
            tc.tile_pool(name="const", bufs=1) as cpool,
            tc.tile_pool(name="dist", bufs=1) as dpool,
            tc.tile_pool(name="scratch", bufs=1) as spool,
            tc.tile_pool(name="work", bufs=1) as wpool,
            tc.tile_pool(name="psum", bufs=1, space="PSUM") as ppool,
        ):
            # dist stream first so the big DMAs start ASAP (sync HWDGE ring);
            # constants ride the scalar HWDGE ring concurrently.
            dtiles = []
            off = 0
            for k, jn in enumerate(SIZES):
                t = dpool.tile([ROWS, jn * D], FP, tag=f"dist{k}")
                nc.sync.dma_start(out=t[:], in_=dist[:, off * D:(off + jn) * D])
                dtiles.append(t)
                off += jn

            cstb_t = cpool.tile([128, CB_TOT], BF, tag="cstb")
            nc.scalar.dma_start(out=cstb_t[:], in_=cstb[:])
            cstf_t = cpool.tile([128, CF_TOT], FP, tag="cstf")
            nc.scalar.dma_start(out=cstf_t[:], in_=cstf[:])
            xT_t = cstb_t[:, CB_XT:CB_XT + ROWS]
            wcf_t = cstb_t[:, CB_WCF:CB_WCF + H]
            wdf_t = cstb_t[:, CB_WDF:CB_WDF + H]
            wfc_t = cstb_t[:, CB_WFC:CB_WFC + D]
            bcf_row = cstb_t[0:1, CB_BCFR:CB_BCFR + H]
            ones_row = cstb_t[0:1, CB_ONES:CB_ONES + ROWS]
            ident = cstf_t[:, CF_EYE:CF_EYE + ROWS]
            bdfN = cstf_t[:, CF_BDFN:CF_BDFN + 1]

            # hx^T = Wcf @ x^T + bcf x ones -> (H, ROWS) in PSUM (bf16 mms)
            hx_ps = ppool.tile([H, ROWS], FP, tag="hx_ps")
            nc.tensor.matmul(hx_ps[:], wcf_t, xT_t, start=True, stop=False)
            nc.tensor.matmul(hx_ps[:], bcf_row, ones_row, start=False, stop=True)
            # fp32 copy for the final DVE mul; bf16 scaled copy for the bias
            # preload term. Both on ACT, reading hx straight from PSUM.
            hxT = wpool.tile([H, ROWS], FP, tag="hxT")
            nc.scalar.copy(hxT[:], hx_ps[:])
            s0T = wpool.tile([H, ROWS], BF, tag="s0T")
            nc.scalar.mul(s0T[:], hx_ps[:], bdfN)

            # Preload the bias term (hx * N*bdf) @ Wfc^T (mirrored: into
            # out^T PSUM); the final matmul accumulates onto it.
            outT_ps = ppool.tile([D, ROWS], FP, tag="outT_ps")
            nc.tensor.matmul(outT_ps[:], wfc_t, s0T[:], start=True, stop=False,
                             skip_group_check=True)

            # Streaming j-reduction: each tile halved with DVE adds (stage 1
            # casts fp32->bf16, mid stages run bf16 at 2x rate, last stage
            # emits fp32 128 cols), then the chunk result is transposed on
            # the PE, accumulating ds^T in PSUM across chunks.
            dsT_ps = ppool.tile([D, ROWS], FP, tag="dsT_ps")
            scs = [spool.tile([ROWS, 64 * max(SIZES)], BF, name=f"sc{i}",
                              tag=f"sc{i}") for i in range(2)]
            rts = [wpool.tile([ROWS, D], FP, name=f"r{i}", tag=f"r{i}")
                   for i in range(2)]
            with nc.allow_low_precision("fold mid-stages in bf16; validated "
                                        "rel err 4e-3 vs 2e-2 budget"):
                for k, jn in enumerate(SIZES):
                    t, sc, r = dtiles[k], scs[k % 2], rts[k % 2]
                    half = jn * D // 2
                    nc.vector.tensor_add(
                        sc[:, 0:half], t[:, 0:half], t[:, half:2 * half]
                    )
                    c = half // 2
                    while c > D:
                        nc.vector.tensor_add(
                            sc[:, 0:c], sc[:, 0:c], sc[:, c:2 * c]
                        )
                        c //= 2
                    nc.vector.tensor_add(r[:], sc[:, 0:D], sc[:, D:2 * D])
                    nc.tensor.matmul(dsT_ps[:], r[:], ident,
                                     is_transpose=True, start=(k == 0),
                                     stop=(k == len(SIZES) - 1),
                                     skip_group_check=True)

            # Tail: dsT -> bf16, hd^T = Wdf @ ds^T, s^T = hx^T * hd^T,
            # out^T += Wfc @ s^T (onto preloaded bias term), store.
            dsT_b = wpool.tile([D, ROWS], BF, tag="dsT_b")
            nc.scalar.copy(dsT_b[:], dsT_ps[:])
            hd_ps = ppool.tile([H, ROWS], FP, tag="hd_ps")
            nc.tensor.matmul(hd_ps[:], wdf_t, dsT_b[:], start=True, stop=True,
                             skip_group_check=True)
            sT = wpool.tile([H, ROWS], BF, tag="sT")
            nc.vector.tensor_mul(sT[:], hd_ps[:], hxT[:])
            nc.tensor.matmul(outT_ps[:], wfc_t, sT[:], start=False, stop=True,
                             skip_group_check=True)
            out_sb = wpool.tile([D, ROWS], FP, tag="out_sb")
            nc.scalar.copy(out_sb[:], outT_ps[:])
            nc.scalar.dma_start(out=out[:], in_=out_sb[:])
    nc.compile()
    return nc


_NC_CACHE = None


def _get_nc():
    global _NC_CACHE
    if _NC_CACHE is None:
        _NC_CACHE = build_nc()
    return _NC_CACHE


def _make_in_maps(x, distance, Wcf_w, Wcf_b, Wdf_w, Wdf_b, Wfc_w):
    x = np.ascontiguousarray(np.asarray(x, np.float32))
    distance = np.ascontiguousarray(np.asarray(distance, np.float32))
    x_flat = x.reshape(B * N, D)
    dist_flat = distance.reshape(B * N, N * D)
    wcfT = np.asarray(Wcf_w, np.float32).T.astype(bfloat16)
    wdfT = np.asarray(Wdf_w, np.float32).T.astype(bfloat16)
    wfcT = np.asarray(Wfc_w, np.float32).T.astype(bfloat16)
    bcf = np.asarray(Wcf_b, np.float32).astype(bfloat16)
    bdfN = (np.asarray(Wdf_b, np.float32) * float(N))
    cstf_blk = np.zeros((128, CF_TOT), np.float32)
    cstf_blk[:, CF_EYE:CF_EYE + ROWS] = np.eye(ROWS, dtype=np.float32)
    cstf_blk[:, CF_BDFN] = bdfN
    in_maps = []
    for c in range(NCORES):
        sl = slice(c * ROWS, (c + 1) * ROWS)
        cstb_blk = np.zeros((128, CB_TOT), bfloat16)
        cstb_blk[:, CB_XT:CB_XT + ROWS] = x_flat[sl].T.astype(bfloat16)
        cstb_blk[:, CB_WCF:CB_WCF + H] = wcfT
        cstb_blk[:, CB_WDF:CB_WDF + H] = wdfT
        cstb_blk[:, CB_WFC:CB_WFC + D] = wfcT
        cstb_blk[0, CB_BCFR:CB_BCFR + H] = bcf
        cstb_blk[0, CB_ONES:CB_ONES + ROWS] = bfloat16(1.0)
        in_maps.append({
            "dist": np.ascontiguousarray(dist_flat[sl]),
            "cstb": cstb_blk,
            "cstf": cstf_blk,
        })
    return in_maps


def kernel(x, distance, Wcf_w, Wcf_b, Wdf_w, Wdf_b, Wfc_w):
    in_maps = _make_in_maps(x, distance, Wcf_w, Wcf_b, Wdf_w, Wdf_b, Wfc_w)
    nc = _get_nc()
    res = run_bass_kernel_spmd(nc, in_maps, list(range(NCORES))).results
    # per-core result is out^T (D, ROWS); transpose back to (ROWS, D)
    out = np.concatenate(
        [np.ascontiguousarray(res[c]["out"].T) for c in range(NCORES)], axis=0
    )
    return out.reshape(B, N, D)
